# revision 5
# baseline (speedup 1.0000x reference)
"""DCN cross-network forward on 8 Trainium2 NeuronCores.

Reference computation (LAYER_NUM=4, INPUT_DIM=1024, BATCH=16384):
    x0 = x
    for i in range(4):
        s  = xi @ w[i]                      # [B] per-row scalar
        xi = x0 * s[:, None] + b[i] + xi

Algebraic collapse: every layer adds a per-row multiple of x0 plus a
constant vector, so
    x_i = alpha_i * x0 + C_i,   C_i = sum_{j<i} b[j]          (constant vec)
    u_i = 1 + x0 . w[i]         (per-row scalars)
    k_i = C_i . w[i]            (host-computable scalar constants)
    alpha_{i+1} = alpha_i * u_i + k_i,  alpha_0 = 1
    out = alpha_4 * x0 + C_4
which reads x exactly once and writes out exactly once (memory roofline).
x and out move as fp16 (gate is rel_err < 2e-2; measured ~1e-3).

Layout: x is pre-transposed on the host into a partition-major blocked
layout (512 rows per block, feature chunks of 128 on partitions):
    xt[blk, p, ch, rr] = x[core*2048 + blk*512 + rr, ch*128 + p]   (fp16)

Schedule (the v1 kernel hit 44us on a 23.5us HBM floor because loads
queued behind compute on the Scalar sequencer and stores bunched at the
end behind a serialized DVE chain):
  - Sync sequencer issues ONLY loads, in program order, all ready at
    t=0: the merged weights tensor first (tiny - everything depends on
    it), then the four x blocks in half-block pieces so dot-product
    matmuls start as soon as the first half lands.
  - Scalar (ACT) owns the two PSUM->SBUF copies per block plus ONLY the
    store dispatches, so a store is never stuck behind another engine's
    wait.  The ones-row for the +1 rank-1 update is embedded in the
    weights tensor (chunk NCH), killing a separate tiny DMA.
  - The alpha recurrence reads fp16 SBUF operands (one ACT copy moves
    all four dot rows PSUM->SBUF in parallel across partitions), so the
    three DVE chain ops run in 16-bit 2x mode instead of fp32-PSUM 1x.
  - Every block's scale multiply is split in halves with the store
    dispatch interleaved, so the store stream starts early and the HBM
    bus stays busy in both directions.

The dots are direct TensorE matmuls (no on-device transpose):
    t[{0,32,64,96}, r] += wt_chunk[128, 97]^T @ xt_chunk[128, 512]
(dot rows land on PSUM partitions 0/32/64/96 - the legal quadrant bases
for 1-partition engine reads - via a zero-padded 97-column stationary
operand; a rank-1 ones matmul adds +1 so PSUM holds u_i directly).
alpha runs in alpha/4096 space (exact power-of-two, undone on host) so
it fits fp16; alpha is broadcast across partitions with a rank-1 ones
matmul into PSUM and one ACT copy applies +k3/4096 while rounding fp16.

Sharding: data-parallel over batch; each of the 8 cores processes a
[2048, 1024] slice with replicated small weights.
"""

import sys

import numpy as np

sys.path.insert(0, "/opt/trn_rl_repo")

BATCH = 16384
D = 1024
L = 4
NCORES = 8
SHARD = BATCH // NCORES  # 2048
P = 128
NCH = D // P             # 8 contraction chunks
F = 512                  # rows per block (PSUM bank limit)
NBLK = SHARD // F        # 4 blocks per core
M = 97                   # padded stationary width (w_i at column 32*i)

_build_cache: dict = {}


def _build_program(k1: float, k2: float, k3: float):
    """Build (and compile) the SPMD Bass program for one core's shard."""
    import concourse.bacc as bacc
    import concourse.mybir as mybir
    import concourse.tile as tile
    f32 = mybir.dt.float32
    f16 = mybir.dt.float16
    mult = mybir.AluOpType.mult
    add = mybir.AluOpType.add
    Copy = mybir.ActivationFunctionType.Copy

    nc = bacc.Bacc("TRN2", target_bir_lowering=False, debug=False)

    xt = nc.dram_tensor("xt", [NBLK, P, NCH, F], f16, kind="ExternalInput").ap()
    # weights + embedded ones-row: chunk NCH partition 0 holds 1.0 at the
    # quadrant columns (the +1 rank-1 stationary), everything else zero.
    wtd = nc.dram_tensor("wtd", [P, NCH + 1, M], f16, kind="ExternalInput").ap()
    out = nc.dram_tensor("out", [NBLK, P, NCH, F], f16, kind="ExternalOutput").ap()

    with tile.TileContext(nc) as tc:
        with (
            tc.tile_pool(name="consts", bufs=1) as cpool,
            tc.tile_pool(name="xin", bufs=4) as xpool,
            tc.tile_pool(name="usb", bufs=2) as upool,
            tc.tile_pool(name="small", bufs=2) as spool,
            tc.tile_pool(name="absb", bufs=2) as abpool,
            tc.tile_pool(name="outp", bufs=4) as opool,
            tc.tile_pool(name="ps_t", bufs=4, space="PSUM") as pst,
            tc.tile_pool(name="ps_ab", bufs=2, space="PSUM") as psab,
        ):
            # ---- loads: everything on the Sync ring, weights first ----
            wt_sb = cpool.tile([P, NCH + 1, M], f16)
            nc.sync.dma_start(out=wt_sb[:], in_=wtd)
            onesF = cpool.tile([1, F], f16)
            nc.vector.memset(onesF[:], 1.0)
            ones128 = cpool.tile([1, P], f16)
            nc.vector.memset(ones128[:], 1.0)

            xbs = []
            for b in range(NBLK):
                xb = xpool.tile([P, NCH, F], f16, tag="x")
                nc.sync.dma_start(out=xb[:, 0:4, :], in_=xt[b, :, 0:4, :])
                nc.sync.dma_start(out=xb[:, 4:8, :], in_=xt[b, :, 4:8, :])
                xbs.append(xb)

            for b in range(NBLK):
                xb = xbs[b]
                # dots: t[32i, r] = sum_d w[i, d]*x[r, d]; +1 via ones rank-1
                tps = pst.tile([P, F], f32, tag="t")
                for c in range(NCH):
                    nc.tensor.matmul(
                        tps[0:M, :],
                        lhsT=wt_sb[:, c, :],
                        rhs=xb[:, c, :],
                        start=(c == 0),
                        stop=False,
                    )
                nc.tensor.matmul(
                    tps[0:M, :], lhsT=wt_sb[0:1, NCH, :], rhs=onesF[:],
                    start=False, stop=True,
                )

                # recurrence: alpha4 = ((u0*u1/4096 + k1')*u2 + k2')*u3 + k3'
                # in0 comes from SBUF, in1 stays in PSUM (the BIR verifier
                # requires equal base partitions when BOTH inputs are SBUF,
                # and the dot rows live at partitions 0/32/64/96)
                u0c = upool.tile([1, F], f32, tag="u0c")
                nc.scalar.copy(out=u0c[:], in_=tps[0:1, :])
                a2 = spool.tile([1, F], f16, tag="a2")
                nc.vector.scalar_tensor_tensor(
                    out=a2[:], in0=u0c[:], scalar=1.0 / 4096.0,
                    in1=tps[32:33, :], op0=mult, op1=mult,
                )
                a3 = spool.tile([1, F], f16, tag="a3")
                nc.vector.scalar_tensor_tensor(
                    out=a3[:], in0=a2[:], scalar=k1 / 4096.0,
                    in1=tps[64:65, :], op0=add, op1=mult,
                )
                a4 = spool.tile([1, F], f16, tag="a4")
                nc.vector.scalar_tensor_tensor(
                    out=a4[:], in0=a3[:], scalar=k2 / 4096.0,
                    in1=tps[96:97, :], op0=add, op1=mult,
                )
                # broadcast alpha/4096 across partitions (fp16 rank-1), then
                # one ACT copy applies +k3/4096 while rounding to fp16
                abp = psab.tile([P, F], f32, tag="abp")
                nc.tensor.matmul(
                    abp[:], lhsT=ones128[:], rhs=a4[:], start=True, stop=True
                )
                ab = abpool.tile([P, 1, F], f16, tag="ab")
                nc.scalar.activation(
                    ab[:, 0, :], abp[:], Copy, bias=k3 / 4096.0, scale=1.0
                )

                # scale + store in halves so the store stream starts early
                ob = opool.tile([P, NCH, F], f16, tag="o")
                nc.vector.tensor_tensor(
                    out=ob[:, 0:4, :], in0=xb[:, 0:4, :],
                    in1=ab[:].to_broadcast([P, 4, F]), op=mult,
                )
                nc.scalar.dma_start(out=out[b, :, 0:4, :], in_=ob[:, 0:4, :])
                nc.vector.tensor_tensor(
                    out=ob[:, 4:8, :], in0=xb[:, 4:8, :],
                    in1=ab[:].to_broadcast([P, 4, F]), op=mult,
                )
                nc.scalar.dma_start(out=out[b, :, 4:8, :], in_=ob[:, 4:8, :])

    nc.compile()
    return nc


def _make_in_maps(x, W):
    """Per-core input maps; x [B, D] fp32, W [L, D] fp32."""
    # xt[core, b, p, ch, r] = x[core*2048 + b*512 + r, ch*128 + p]
    # (partition-major: each SBUF partition line is one contiguous 8KB)
    xt = np.ascontiguousarray(
        x.reshape(NCORES, NBLK, F, NCH, P).transpose(0, 1, 4, 3, 2)
    ).astype(np.float16)
    wt = np.zeros((P, NCH + 1, M), dtype=np.float16)
    wt[:, :NCH, ::32] = W.reshape(L, NCH, P).transpose(2, 1, 0)
    wt[0, NCH, ::32] = 1.0  # embedded ones-row for the +1 rank-1 update
    return [{"xt": xt[c], "wtd": wt} for c in range(NCORES)]


def kernel(x, cross_weights, cross_bias):
    from concourse.bass_utils import run_bass_kernel_spmd

    x = np.ascontiguousarray(np.asarray(x, dtype=np.float32))
    W = np.ascontiguousarray(np.asarray(cross_weights, dtype=np.float32))
    Bb = np.asarray(cross_bias, dtype=np.float32)
    assert x.shape == (BATCH, D) and W.shape == (L, D) and Bb.shape == (L, D)

    # host-side scalar constants k_i = C_i . w_i with C_i = sum_{j<i} b_j
    C = np.zeros(D, dtype=np.float32)
    ks = []
    for i in range(L):
        ks.append(float(C @ W[i]))
        C = C + Bb[i]
    # ks[0] == 0 always (C_0 = 0); bake the other three
    k1, k2, k3 = ks[1], ks[2], ks[3]

    key = (k1, k2, k3)
    nc = _build_cache.get(key)
    if nc is None:
        nc = _build_program(k1, k2, k3)
        _build_cache[key] = nc

    in_maps = _make_in_maps(x, W)
    res = run_bass_kernel_spmd(nc, in_maps, list(range(NCORES)))
    # invert the transposed layout: full[core*2048 + b*512 + r, c*128 + p]
    stacked = np.stack(
        [np.asarray(res.results[c]["out"]) for c in range(NCORES)], axis=0
    ).astype(np.float32)  # [core, b, p, ch, F]
    stacked *= 4096.0  # undo the device-side 1/4096 alpha pre-scale
    full = np.ascontiguousarray(
        stacked.transpose(0, 1, 4, 3, 2).reshape(BATCH, D)
    )
    full += C[None, :]  # C_4 broadcast-add on host
    return full


# revision 10
# speedup vs baseline: 1.1768x; 1.1768x over previous
"""DCN cross-network forward on 8 Trainium2 NeuronCores.

Reference computation (LAYER_NUM=4, INPUT_DIM=1024, BATCH=16384):
    x0 = x
    for i in range(4):
        s  = xi @ w[i]                      # [B] per-row scalar
        xi = x0 * s[:, None] + b[i] + xi

Algebraic collapse: every layer adds a per-row multiple of x0 plus a
constant vector, so
    x_i = alpha_i * x0 + C_i,   C_i = sum_{j<i} b[j]          (constant vec)
    u_i = 1 + x0 . w[i]         (per-row scalars)
    k_i = C_i . w[i]            (host-computable scalar constants)
    alpha_{i+1} = alpha_i * u_i + k_i,  alpha_0 = 1
    out = alpha_4 * x0 + C_4
which reads x exactly once and writes out exactly once (memory roofline).
x and out move as fp16 (gate is rel_err < 2e-2; measured ~1e-3).

Layout: x is pre-transposed on the host into a partition-major blocked
layout (512 rows per block, feature chunks of 128 on partitions):
    xt[blk, p, ch, rr] = x[core*2048 + blk*512 + rr, ch*128 + p]   (fp16)

Schedule (the v1 kernel hit 44us on a 23.5us HBM floor because loads
queued behind compute on the Scalar sequencer and stores bunched at the
end behind a serialized DVE chain):
  - Sync sequencer issues ONLY loads, in program order, all ready at
    t=0: the merged weights tensor first (tiny - everything depends on
    it), then the four x blocks in half-block pieces so dot-product
    matmuls start as soon as the first half lands.
  - Scalar (ACT) owns the two PSUM->SBUF copies per block plus ONLY the
    store dispatches, so a store is never stuck behind another engine's
    wait.  The ones-row for the +1 rank-1 update is embedded in the
    weights tensor (chunk NCH), killing a separate tiny DMA.
  - The alpha recurrence reads fp16 SBUF operands (one ACT copy moves
    all four dot rows PSUM->SBUF in parallel across partitions), so the
    three DVE chain ops run in 16-bit 2x mode instead of fp32-PSUM 1x.
  - Every block's scale multiply is split in halves with the store
    dispatch interleaved, so the store stream starts early and the HBM
    bus stays busy in both directions.

The dots are direct TensorE matmuls (no on-device transpose):
    t[{0,32,64,96}, r] += wt_chunk[128, 97]^T @ xt_chunk[128, 512]
(dot rows land on PSUM partitions 0/32/64/96 - the legal quadrant bases
for 1-partition engine reads - via a zero-padded 97-column stationary
operand; a rank-1 ones matmul adds +1 so PSUM holds u_i directly).
alpha runs in alpha/4096 space (exact power-of-two, undone on host) so
it fits fp16; alpha is broadcast across partitions with a rank-1 ones
matmul into PSUM and one ACT copy applies +k3/4096 while rounding fp16.

Sharding: data-parallel over batch; each of the 8 cores processes a
[2048, 1024] slice with replicated small weights.
"""

import sys

import numpy as np

sys.path.insert(0, "/opt/trn_rl_repo")

BATCH = 16384
D = 1024
L = 4
NCORES = 8
SHARD = BATCH // NCORES  # 2048
P = 128
NCH = D // P             # 8 contraction chunks
F = 512                  # rows per block (PSUM bank limit)
NBLK = SHARD // F        # 4 blocks per core
M = 97                   # padded stationary width (w_i at column 32*i)

_build_cache: dict = {}


def _build_program(k1: float, k2: float, k3: float):
    """Build (and compile) the SPMD Bass program for one core's shard."""
    import concourse.bacc as bacc
    import concourse.mybir as mybir
    import concourse.tile as tile
    f32 = mybir.dt.float32
    f16 = mybir.dt.float16
    mult = mybir.AluOpType.mult
    add = mybir.AluOpType.add
    Copy = mybir.ActivationFunctionType.Copy

    nc = bacc.Bacc("TRN2", target_bir_lowering=False, debug=False)

    # half-blocks are separately contiguous in DRAM (4KB partition lines
    # back to back) so a half-load is one fully-contiguous 512KB DMA
    xt = nc.dram_tensor("xt", [NBLK, 2, P, NCH // 2, F], f16, kind="ExternalInput").ap()
    # weights + embedded ones-row: chunk NCH partition 0 holds 1.0 at the
    # quadrant columns (the +1 rank-1 stationary), everything else zero.
    wtd = nc.dram_tensor("wtd", [P, NCH + 1, M], f16, kind="ExternalInput").ap()
    out = nc.dram_tensor("out", [NBLK, 2, P, NCH // 2, F], f16, kind="ExternalOutput").ap()

    with tile.TileContext(nc) as tc:
        with (
            tc.tile_pool(name="consts", bufs=1) as cpool,
            tc.tile_pool(name="xin", bufs=4) as xpool,
            tc.tile_pool(name="usb", bufs=2) as upool,
            tc.tile_pool(name="small", bufs=2) as spool,
            tc.tile_pool(name="absb", bufs=2) as abpool,
            tc.tile_pool(name="outp", bufs=4) as opool,
            tc.tile_pool(name="ps_t", bufs=4, space="PSUM") as pst,
            tc.tile_pool(name="ps_ab", bufs=2, space="PSUM") as psab,
        ):
            # ---- loads: weights on the Scalar ring (lands in ~2us while
            # the x stream starts on Sync), x half-blocks on Sync only ----
            wt_sb = cpool.tile([P, NCH + 1, M], f16)
            nc.scalar.dma_start(out=wt_sb[:], in_=wtd)
            onesF = cpool.tile([1, F], f16)
            nc.vector.memset(onesF[:], 1.0)
            ones128 = cpool.tile([1, P], f16)
            nc.vector.memset(ones128[:], 1.0)

            xbs = []
            for b in range(NBLK):
                xb = xpool.tile([P, NCH, F], f16, tag="x")
                nc.sync.dma_start(out=xb[:, 0:4, :], in_=xt[b, 0])
                nc.sync.dma_start(out=xb[:, 4:8, :], in_=xt[b, 1])
                xbs.append(xb)

            for b in range(NBLK):
                xb = xbs[b]
                # dots: t[32i, r] = sum_d w[i, d]*x[r, d]; +1 via ones rank-1
                tps = pst.tile([P, F], f32, tag="t")
                for c in range(NCH):
                    nc.tensor.matmul(
                        tps[0:M, :],
                        lhsT=wt_sb[:, c, :],
                        rhs=xb[:, c, :],
                        start=(c == 0),
                        stop=False,
                    )
                nc.tensor.matmul(
                    tps[0:M, :], lhsT=wt_sb[0:1, NCH, :], rhs=onesF[:],
                    start=False, stop=True,
                )

                # A sim-time floor per block keeps the scheduler from
                # software-pipelining block b+1's chain ops AHEAD of block
                # b's in the in-order engine queues (its DMA model is too
                # optimistic about when later blocks land, and the
                # resulting head-of-line blocking cascades).
                with tc.tile_wait_until(ms=0.004 * b):
                    # recurrence: alpha4 = ((u0*u1/4096 + k1')*u2 + k2')*u3
                    # + k3'.  in0 from SBUF, in1 stays in PSUM (the BIR
                    # verifier requires equal base partitions when BOTH
                    # inputs are SBUF; dot rows live at partitions
                    # 0/32/64/96)
                    u0c = upool.tile([1, F], f32, tag="u0c")
                    nc.scalar.copy(out=u0c[:], in_=tps[0:1, :])
                    a2 = spool.tile([1, F], f16, tag="a2")
                    nc.vector.scalar_tensor_tensor(
                        out=a2[:], in0=u0c[:], scalar=1.0 / 4096.0,
                        in1=tps[32:33, :], op0=mult, op1=mult,
                    )
                    a3 = spool.tile([1, F], f16, tag="a3")
                    nc.vector.scalar_tensor_tensor(
                        out=a3[:], in0=a2[:], scalar=k1 / 4096.0,
                        in1=tps[64:65, :], op0=add, op1=mult,
                    )
                    a4 = spool.tile([1, F], f16, tag="a4")
                    nc.vector.scalar_tensor_tensor(
                        out=a4[:], in0=a3[:], scalar=k2 / 4096.0,
                        in1=tps[96:97, :], op0=add, op1=mult,
                    )
                    # broadcast alpha/4096 across partitions (fp16 rank-1),
                    # then one ACT copy applies +k3/4096, rounding to fp16
                    abp = psab.tile([P, F], f32, tag="abp")
                    nc.tensor.matmul(
                        abp[:], lhsT=ones128[:], rhs=a4[:], start=True,
                        stop=True,
                    )
                    ab = abpool.tile([P, 1, F], f16, tag="ab")
                    nc.scalar.activation(
                        ab[:, 0, :], abp[:], Copy, bias=k3 / 4096.0, scale=1.0
                    )

                    # scale + store in halves so the store stream starts
                    # early and stays fed
                    ob = opool.tile([P, NCH, F], f16, tag="o")
                    nc.vector.tensor_tensor(
                        out=ob[:, 0:4, :], in0=xb[:, 0:4, :],
                        in1=ab[:].to_broadcast([P, 4, F]), op=mult,
                    )
                    nc.scalar.dma_start(out=out[b, 0], in_=ob[:, 0:4, :])
                    nc.vector.tensor_tensor(
                        out=ob[:, 4:8, :], in0=xb[:, 4:8, :],
                        in1=ab[:].to_broadcast([P, 4, F]), op=mult,
                    )
                    nc.scalar.dma_start(out=out[b, 1], in_=ob[:, 4:8, :])

    nc.compile()
    return nc


def _make_in_maps(x, W):
    """Per-core input maps; x [B, D] fp32, W [L, D] fp32."""
    # xt[core, b, h, p, ch, r] = x[core*2048 + b*512 + r, (4h+ch)*128 + p]
    # (partition-major, and each half-block is one contiguous 512KB run)
    xt = np.ascontiguousarray(
        x.reshape(NCORES, NBLK, F, 2, NCH // 2, P).transpose(0, 1, 3, 5, 4, 2)
    ).astype(np.float16)
    wt = np.zeros((P, NCH + 1, M), dtype=np.float16)
    wt[:, :NCH, ::32] = W.reshape(L, NCH, P).transpose(2, 1, 0)
    wt[0, NCH, ::32] = 1.0  # embedded ones-row for the +1 rank-1 update
    return [{"xt": xt[c], "wtd": wt} for c in range(NCORES)]


def kernel(x, cross_weights, cross_bias):
    from concourse.bass_utils import run_bass_kernel_spmd

    x = np.ascontiguousarray(np.asarray(x, dtype=np.float32))
    W = np.ascontiguousarray(np.asarray(cross_weights, dtype=np.float32))
    Bb = np.asarray(cross_bias, dtype=np.float32)
    assert x.shape == (BATCH, D) and W.shape == (L, D) and Bb.shape == (L, D)

    # host-side scalar constants k_i = C_i . w_i with C_i = sum_{j<i} b_j
    C = np.zeros(D, dtype=np.float32)
    ks = []
    for i in range(L):
        ks.append(float(C @ W[i]))
        C = C + Bb[i]
    # ks[0] == 0 always (C_0 = 0); bake the other three
    k1, k2, k3 = ks[1], ks[2], ks[3]

    key = (k1, k2, k3)
    nc = _build_cache.get(key)
    if nc is None:
        nc = _build_program(k1, k2, k3)
        _build_cache[key] = nc

    in_maps = _make_in_maps(x, W)
    res = run_bass_kernel_spmd(nc, in_maps, list(range(NCORES)))
    # invert the transposed layout:
    #   full[core*2048 + b*512 + r, (4h+c)*128 + p] = out[core, b, h, p, c, r]
    stacked = np.stack(
        [np.asarray(res.results[c]["out"]) for c in range(NCORES)], axis=0
    ).astype(np.float32)  # [core, b, h, p, ch, F]
    stacked *= 4096.0  # undo the device-side 1/4096 alpha pre-scale
    full = np.ascontiguousarray(
        stacked.transpose(0, 1, 5, 2, 4, 3).reshape(BATCH, D)
    )
    full += C[None, :]  # C_4 broadcast-add on host
    return full


# revision 20
# speedup vs baseline: 1.2027x; 1.0220x over previous
"""DCN cross-network forward on 8 Trainium2 NeuronCores.

Reference computation (LAYER_NUM=4, INPUT_DIM=1024, BATCH=16384):
    x0 = x
    for i in range(4):
        s  = xi @ w[i]                      # [B] per-row scalar
        xi = x0 * s[:, None] + b[i] + xi

Algebraic collapse: every layer adds a per-row multiple of x0 plus a
constant vector, so
    x_i = alpha_i * x0 + C_i,   C_i = sum_{j<i} b[j]          (constant vec)
    u_i = 1 + x0 . w[i]         (per-row scalars)
    k_i = C_i . w[i]            (host-computable scalar constants)
    alpha_{i+1} = alpha_i * u_i + k_i,  alpha_0 = 1
    out = alpha_4 * x0 + C_4
which reads x exactly once and writes out exactly once (memory roofline).
x and out move as fp16 (gate is rel_err < 2e-2; measured ~1e-3).

Layout: x is pre-transposed on the host into a partition-major blocked
layout (512 rows per block, feature chunks of 128 on partitions):
    xt[blk, p, ch, rr] = x[core*2048 + blk*512 + rr, ch*128 + p]   (fp16)

Schedule (the v1 kernel hit 44us on a 23.5us HBM floor because loads
queued behind compute on the Scalar sequencer and stores bunched at the
end behind a serialized DVE chain):
  - Sync sequencer issues ONLY loads, in program order, all ready at
    t=0: the merged weights tensor first (tiny - everything depends on
    it), then the four x blocks in half-block pieces so dot-product
    matmuls start as soon as the first half lands.
  - Scalar (ACT) owns the two PSUM->SBUF copies per block plus ONLY the
    store dispatches, so a store is never stuck behind another engine's
    wait.  The ones-row for the +1 rank-1 update is embedded in the
    weights tensor (chunk NCH), killing a separate tiny DMA.
  - The alpha recurrence reads fp16 SBUF operands (one ACT copy moves
    all four dot rows PSUM->SBUF in parallel across partitions), so the
    three DVE chain ops run in 16-bit 2x mode instead of fp32-PSUM 1x.
  - Every block's scale multiply is split in halves with the store
    dispatch interleaved, so the store stream starts early and the HBM
    bus stays busy in both directions.

The dots are direct TensorE matmuls (no on-device transpose):
    t[{0,32,64,96}, r] += wt_chunk[128, 97]^T @ xt_chunk[128, 512]
(dot rows land on PSUM partitions 0/32/64/96 - the legal quadrant bases
for 1-partition engine reads - via a zero-padded 97-column stationary
operand; a rank-1 ones matmul adds +1 so PSUM holds u_i directly).
alpha runs in alpha/4096 space (exact power-of-two, undone on host) so
it fits fp16; alpha is broadcast across partitions with a rank-1 ones
matmul into PSUM and one ACT copy applies +k3/4096 while rounding fp16.

Sharding: data-parallel over batch; each of the 8 cores processes a
[2048, 1024] slice with replicated small weights.
"""

import sys

import numpy as np

sys.path.insert(0, "/opt/trn_rl_repo")

BATCH = 16384
D = 1024
L = 4
NCORES = 8
SHARD = BATCH // NCORES  # 2048
P = 128
NCH = D // P             # 8 contraction chunks
F = 512                  # rows per block (PSUM bank limit)
NBLK = SHARD // F        # 4 blocks per core
M = 97                   # padded stationary width (w_i at column 32*i)

_build_cache: dict = {}


def _build_program(k1: float, k2: float, k3: float):
    """Build (and compile) the SPMD Bass program for one core's shard."""
    import concourse.bacc as bacc
    import concourse.mybir as mybir
    import concourse.tile as tile
    f32 = mybir.dt.float32
    f16 = mybir.dt.float16
    mult = mybir.AluOpType.mult
    add = mybir.AluOpType.add
    Copy = mybir.ActivationFunctionType.Copy

    nc = bacc.Bacc("TRN2", target_bir_lowering=False, debug=False)

    # half-blocks are separately contiguous in DRAM (4KB partition lines
    # back to back) so a half-load is one fully-contiguous 512KB DMA
    xt = nc.dram_tensor("xt", [NBLK, 2, P, NCH // 2, F], f16, kind="ExternalInput").ap()
    # weights + embedded ones-row: chunk NCH partition 0 holds 1.0 at the
    # quadrant columns (the +1 rank-1 stationary), everything else zero.
    wtd = nc.dram_tensor("wtd", [P, NCH + 1, M], f16, kind="ExternalInput").ap()
    idn = nc.dram_tensor("idn", [P, P], f16, kind="ExternalInput").ap()
    out = nc.dram_tensor("out", [NBLK, 2, P, NCH // 2, F], f16, kind="ExternalOutput").ap()

    with tile.TileContext(nc) as tc:
        with (
            tc.tile_pool(name="consts", bufs=1) as cpool,
            tc.tile_pool(name="xin", bufs=4) as xpool,
            tc.tile_pool(name="usb", bufs=2) as upool,
            tc.tile_pool(name="small", bufs=2) as spool,
            tc.tile_pool(name="absb", bufs=2) as abpool,
            tc.tile_pool(name="outp", bufs=4) as opool,
            tc.tile_pool(name="ps_t", bufs=2, space="PSUM") as pst,
            tc.tile_pool(name="ps_tT", bufs=2, space="PSUM") as pstT,
            tc.tile_pool(name="ps_aT", bufs=2, space="PSUM") as psaT,
            tc.tile_pool(name="ps_ab", bufs=2, space="PSUM") as psab,
        ):
            # ---- loads: weights on the Scalar ring (lands in ~2us while
            # the x stream starts on Sync), x half-blocks on Sync only ----
            wt_sb = cpool.tile([P, NCH + 1, M], f16)
            nc.scalar.dma_start(out=wt_sb[:], in_=wtd)
            id_sb = cpool.tile([P, P], f16)
            nc.scalar.dma_start(out=id_sb[:], in_=idn)
            onesF = cpool.tile([1, F], f16)
            nc.vector.memset(onesF[:], 1.0)
            ones128 = cpool.tile([1, P], f16)
            nc.vector.memset(ones128[:], 1.0)

            xbs = []
            for b in range(NBLK):
                xb = xpool.tile([P, NCH, F], f16, tag="x")
                nc.sync.dma_start(out=xb[:, 0:4, :], in_=xt[b, 0])
                nc.sync.dma_start(out=xb[:, 4:8, :], in_=xt[b, 1])
                xbs.append(xb)

            for b in range(NBLK):
                xb = xbs[b]
                # dots: t[32i, r] = sum_d w[i, d]*x[r, d]; +1 via ones rank-1
                tps = pst.tile([P, F], f32, tag="t")
                for c in range(NCH):
                    nc.tensor.matmul(
                        tps[0:M, :],
                        lhsT=wt_sb[:, c, :],
                        rhs=xb[:, c, :],
                        start=(c == 0),
                        stop=False,
                    )
                nc.tensor.matmul(
                    tps[0:M, :], lhsT=wt_sb[0:1, NCH, :], rhs=onesF[:],
                    start=False, stop=True,
                )

                # A sim-time floor per block keeps the scheduler from
                # software-pipelining block b+1's chain ops AHEAD of block
                # b's in the in-order engine queues (its DMA model is too
                # optimistic about when later blocks land, and the
                # resulting head-of-line blocking cascades).
                with tc.tile_wait_until(ms=0.004 * b):
                    # Move the dot rows off the single-lane form: one ACT
                    # copy brings rows 0..96 PSUM->SBUF fp16 (parallel
                    # across partitions), then four PE transposes put the
                    # rows on r-major layout tpsT[r_sub, g, 32i] so the
                    # recurrence runs as [128, 4] ops on 128 DVE lanes
                    # instead of [1, 512] ops on one lane.
                    usb = upool.tile([P, F], f16, tag="u")
                    nc.scalar.copy(out=usb[0:M, :], in_=tps[0:M, :])
                    # per-group stride padded to 98 fp16 (196B) to keep each
                    # transpose's PSUM base 4-byte aligned
                    tpsT = pstT.tile([P, 4, M + 1], f16, tag="tT")
                    for g in range(4):
                        nc.tensor.transpose(
                            tpsT[:, g, 0:M],
                            usb[0:M, g * P:(g + 1) * P],
                            id_sb[0:M, 0:M],
                        )
                    # recurrence: alpha4 = ((u0*u1/4096 + k1')*u2 + k2')*u3
                    # + k3' on [128, 4] slices (both PSUM operands share
                    # base partition 0, so the verifier is happy)
                    u0sb = spool.tile([P, 4], f16, tag="u0")
                    nc.vector.tensor_copy(u0sb[:], tpsT[:, :, 0])
                    a2 = spool.tile([P, 4], f16, tag="a2")
                    nc.vector.scalar_tensor_tensor(
                        out=a2[:], in0=u0sb[:], scalar=1.0 / 4096.0,
                        in1=tpsT[:, :, 32], op0=mult, op1=mult,
                    )
                    a3 = spool.tile([P, 4], f16, tag="a3")
                    nc.vector.scalar_tensor_tensor(
                        out=a3[:], in0=a2[:], scalar=k1 / 4096.0,
                        in1=tpsT[:, :, 64], op0=add, op1=mult,
                    )
                    a4 = spool.tile([P, 4], f16, tag="a4")
                    nc.vector.scalar_tensor_tensor(
                        out=a4[:], in0=a3[:], scalar=k2 / 4096.0,
                        in1=tpsT[:, :, 96], op0=add, op1=mult,
                    )
                    # transpose alpha back to one partition ([128,1] -> a
                    # [1,128] slice each), one ACT copy to fp16 SBUF, then
                    # the rank-1 ones broadcast as before
                    aTp = psaT.tile([1, F], f16, tag="aT")
                    for g in range(4):
                        nc.tensor.transpose(
                            aTp[0:1, g * P:(g + 1) * P], a4[:, g:g + 1],
                            id_sb[:],
                        )
                    a4T = abpool.tile([1, F], f16, tag="a4T")
                    nc.scalar.copy(out=a4T[:], in_=aTp[:])
                    abp = psab.tile([P, F], f32, tag="abp")
                    nc.tensor.matmul(
                        abp[:], lhsT=ones128[:], rhs=a4T[:], start=True,
                        stop=True,
                    )
                    ab = abpool.tile([P, 1, F], f16, tag="ab")
                    nc.scalar.activation(
                        ab[:, 0, :], abp[:], Copy, bias=k3 / 4096.0, scale=1.0
                    )

                    # scale + store in halves so the store stream starts
                    # early and stays fed
                    ob = opool.tile([P, NCH, F], f16, tag="o")
                    nc.vector.tensor_tensor(
                        out=ob[:, 0:4, :], in0=xb[:, 0:4, :],
                        in1=ab[:].to_broadcast([P, 4, F]), op=mult,
                    )
                    nc.scalar.dma_start(out=out[b, 0], in_=ob[:, 0:4, :])
                    nc.vector.tensor_tensor(
                        out=ob[:, 4:8, :], in0=xb[:, 4:8, :],
                        in1=ab[:].to_broadcast([P, 4, F]), op=mult,
                    )
                    nc.scalar.dma_start(out=out[b, 1], in_=ob[:, 4:8, :])

    nc.compile()
    return nc


def _make_in_maps(x, W):
    """Per-core input maps; x [B, D] fp32, W [L, D] fp32."""
    # xt[core, b, h, p, ch, r] = x[core*2048 + b*512 + r, (4h+ch)*128 + p]
    # (partition-major, and each half-block is one contiguous 512KB run)
    xt = np.ascontiguousarray(
        x.reshape(NCORES, NBLK, F, 2, NCH // 2, P).transpose(0, 1, 3, 5, 4, 2)
    ).astype(np.float16)
    wt = np.zeros((P, NCH + 1, M), dtype=np.float16)
    wt[:, :NCH, ::32] = W.reshape(L, NCH, P).transpose(2, 1, 0)
    wt[0, NCH, ::32] = 1.0  # embedded ones-row for the +1 rank-1 update
    idn = np.eye(P, dtype=np.float16)  # PE-transpose identity
    return [{"xt": xt[c], "wtd": wt, "idn": idn} for c in range(NCORES)]


def kernel(x, cross_weights, cross_bias):
    from concourse.bass_utils import run_bass_kernel_spmd

    x = np.ascontiguousarray(np.asarray(x, dtype=np.float32))
    W = np.ascontiguousarray(np.asarray(cross_weights, dtype=np.float32))
    Bb = np.asarray(cross_bias, dtype=np.float32)
    assert x.shape == (BATCH, D) and W.shape == (L, D) and Bb.shape == (L, D)

    # host-side scalar constants k_i = C_i . w_i with C_i = sum_{j<i} b_j
    C = np.zeros(D, dtype=np.float32)
    ks = []
    for i in range(L):
        ks.append(float(C @ W[i]))
        C = C + Bb[i]
    # ks[0] == 0 always (C_0 = 0); bake the other three
    k1, k2, k3 = ks[1], ks[2], ks[3]

    key = (k1, k2, k3)
    nc = _build_cache.get(key)
    if nc is None:
        nc = _build_program(k1, k2, k3)
        _build_cache[key] = nc

    in_maps = _make_in_maps(x, W)
    res = run_bass_kernel_spmd(nc, in_maps, list(range(NCORES)))
    # invert the transposed layout:
    #   full[core*2048 + b*512 + r, (4h+c)*128 + p] = out[core, b, h, p, c, r]
    stacked = np.stack(
        [np.asarray(res.results[c]["out"]) for c in range(NCORES)], axis=0
    ).astype(np.float32)  # [core, b, h, p, ch, F]
    stacked *= 4096.0  # undo the device-side 1/4096 alpha pre-scale
    full = np.ascontiguousarray(
        stacked.transpose(0, 1, 5, 2, 4, 3).reshape(BATCH, D)
    )
    full += C[None, :]  # C_4 broadcast-add on host
    return full


# revision 26
# speedup vs baseline: 1.2108x; 1.0067x over previous
"""DCN cross-network forward on 8 Trainium2 NeuronCores.

Reference computation (LAYER_NUM=4, INPUT_DIM=1024, BATCH=16384):
    x0 = x
    for i in range(4):
        s  = xi @ w[i]                      # [B] per-row scalar
        xi = x0 * s[:, None] + b[i] + xi

Algebraic collapse: every layer adds a per-row multiple of x0 plus a
constant vector, so
    x_i = alpha_i * x0 + C_i,   C_i = sum_{j<i} b[j]          (constant vec)
    u_i = 1 + x0 . w[i]         (per-row scalars)
    k_i = C_i . w[i]            (host-computable scalar constants)
    alpha_{i+1} = alpha_i * u_i + k_i,  alpha_0 = 1
    out = alpha_4 * x0 + C_4
which reads x exactly once and writes out exactly once (memory roofline).
x and out move as fp16 (gate is rel_err < 2e-2; measured ~1e-3).

Layout: x is pre-transposed on the host into a partition-major blocked
layout (512 rows per block, feature chunks of 128 on partitions):
    xt[blk, p, ch, rr] = x[core*2048 + blk*512 + rr, ch*128 + p]   (fp16)

Schedule (the v1 kernel hit 44us on a 23.5us HBM floor because loads
queued behind compute on the Scalar sequencer and stores bunched at the
end behind a serialized DVE chain):
  - Sync sequencer issues ONLY loads, in program order, all ready at
    t=0: the merged weights tensor first (tiny - everything depends on
    it), then the four x blocks in half-block pieces so dot-product
    matmuls start as soon as the first half lands.
  - Scalar (ACT) owns the two PSUM->SBUF copies per block plus ONLY the
    store dispatches, so a store is never stuck behind another engine's
    wait.  The ones-row for the +1 rank-1 update is embedded in the
    weights tensor (chunk NCH), killing a separate tiny DMA.
  - The alpha recurrence reads fp16 SBUF operands (one ACT copy moves
    all four dot rows PSUM->SBUF in parallel across partitions), so the
    three DVE chain ops run in 16-bit 2x mode instead of fp32-PSUM 1x.
  - Every block's scale multiply is split in halves with the store
    dispatch interleaved, so the store stream starts early and the HBM
    bus stays busy in both directions.

The dots are direct TensorE matmuls (no on-device transpose):
    t[{0,32,64,96}, r] += wt_chunk[128, 97]^T @ xt_chunk[128, 512]
(dot rows land on PSUM partitions 0/32/64/96 - the legal quadrant bases
for 1-partition engine reads - via a zero-padded 97-column stationary
operand; a rank-1 ones matmul adds +1 so PSUM holds u_i directly).
alpha runs in alpha/4096 space (exact power-of-two, undone on host) so
it fits fp16; alpha is broadcast across partitions with a rank-1 ones
matmul into PSUM and one ACT copy applies +k3/4096 while rounding fp16.

Sharding: data-parallel over batch; each of the 8 cores processes a
[2048, 1024] slice with replicated small weights.
"""

import sys

import numpy as np

sys.path.insert(0, "/opt/trn_rl_repo")

BATCH = 16384
D = 1024
L = 4
NCORES = 8
SHARD = BATCH // NCORES  # 2048
P = 128
NCH = D // P             # 8 contraction chunks
F = 512                  # rows per block (PSUM bank limit)
NBLK = SHARD // F        # 4 blocks per core
M = 97                   # padded stationary width (w_i at column 32*i)

_build_cache: dict = {}


def _build_program(k1: float, k2: float, k3: float):
    """Build (and compile) the SPMD Bass program for one core's shard."""
    import concourse.bacc as bacc
    import concourse.mybir as mybir
    import concourse.tile as tile
    f32 = mybir.dt.float32
    f16 = mybir.dt.float16
    mult = mybir.AluOpType.mult
    add = mybir.AluOpType.add
    Copy = mybir.ActivationFunctionType.Copy

    nc = bacc.Bacc("TRN2", target_bir_lowering=False, debug=False)

    # half-blocks are separately contiguous in DRAM (4KB partition lines
    # back to back) so a half-load is one fully-contiguous 512KB DMA
    xt = nc.dram_tensor("xt", [NBLK, 2, P, NCH // 2, F], f16, kind="ExternalInput").ap()
    # weights + embedded ones-row: chunk NCH partition 0 holds 1.0 at the
    # quadrant columns (the +1 rank-1 stationary), everything else zero.
    wtd = nc.dram_tensor("wtd", [P, NCH + 1, M], f16, kind="ExternalInput").ap()
    idn = nc.dram_tensor("idn", [P, P], f16, kind="ExternalInput").ap()
    out = nc.dram_tensor("out", [NBLK, 2, P, NCH // 2, F], f16, kind="ExternalOutput").ap()

    with tile.TileContext(nc) as tc:
        with (
            tc.tile_pool(name="consts", bufs=1) as cpool,
            tc.tile_pool(name="xin", bufs=4) as xpool,
            tc.tile_pool(name="usb", bufs=2) as upool,
            tc.tile_pool(name="small", bufs=2) as spool,
            tc.tile_pool(name="absb", bufs=2) as abpool,
            tc.tile_pool(name="outp", bufs=4) as opool,
            tc.tile_pool(name="ps_t", bufs=2, space="PSUM") as pst,
            tc.tile_pool(name="ps_tT", bufs=2, space="PSUM") as pstT,
            tc.tile_pool(name="ps_aT", bufs=2, space="PSUM") as psaT,
            tc.tile_pool(name="ps_ab", bufs=2, space="PSUM") as psab,
        ):
            # ---- loads: weights on the Scalar ring (lands in ~2us while
            # the x stream starts on Sync), x half-blocks on Sync only ----
            wt_sb = cpool.tile([P, NCH + 1, M], f16)
            nc.scalar.dma_start(out=wt_sb[:], in_=wtd)
            id_sb = cpool.tile([P, P], f16)
            nc.scalar.dma_start(out=id_sb[:], in_=idn)
            onesF = cpool.tile([1, F], f16)
            nc.vector.memset(onesF[:], 1.0)
            ones128 = cpool.tile([1, P], f16)
            nc.vector.memset(ones128[:], 1.0)

            xbs = []
            for b in range(NBLK):
                xb = xpool.tile([P, NCH, F], f16, tag="x")
                nc.sync.dma_start(out=xb[:, 0:4, :], in_=xt[b, 0])
                nc.sync.dma_start(out=xb[:, 4:8, :], in_=xt[b, 1])
                xbs.append(xb)

            for b in range(NBLK):
                xb = xbs[b]
                # dots: t[32i, r] = sum_d w[i, d]*x[r, d]; +1 via ones rank-1
                tps = pst.tile([P, F], f32, tag="t")
                for c in range(NCH):
                    nc.tensor.matmul(
                        tps[0:M, :],
                        lhsT=wt_sb[:, c, :],
                        rhs=xb[:, c, :],
                        start=(c == 0),
                        stop=False,
                    )
                nc.tensor.matmul(
                    tps[0:M, :], lhsT=wt_sb[0:1, NCH, :], rhs=onesF[:],
                    start=False, stop=True,
                )

                # A sim-time floor per block keeps the scheduler from
                # software-pipelining block b+1's chain ops AHEAD of block
                # b's in the in-order engine queues (its DMA model is too
                # optimistic about when later blocks land, and the
                # resulting head-of-line blocking cascades).
                with tc.tile_wait_until(ms=0.004 * b):
                    # Move the dot rows off the single-lane form: one ACT
                    # copy brings rows 0..96 PSUM->SBUF fp16 (parallel
                    # across partitions), then four PE transposes put the
                    # rows on r-major layout tpsT[r_sub, g, 32i] so the
                    # recurrence runs as [128, 4] ops on 128 DVE lanes
                    # instead of [1, 512] ops on one lane.
                    usb = upool.tile([P, F], f16, tag="u")
                    nc.scalar.copy(out=usb[0:M, :], in_=tps[0:M, :])
                    # per-group stride padded to 98 fp16 (196B) to keep each
                    # transpose's PSUM base 4-byte aligned
                    tpsT = pstT.tile([P, 4, M + 1], f16, tag="tT")
                    for g in range(4):
                        nc.tensor.transpose(
                            tpsT[:, g, 0:M],
                            usb[0:M, g * P:(g + 1) * P],
                            id_sb[0:M, 0:M],
                        )
                    # recurrence: alpha4 = ((u0*u1/4096 + k1')*u2 + k2')*u3
                    # + k3' on [128, 4] slices (both PSUM operands share
                    # base partition 0, so the verifier is happy)
                    u0sb = spool.tile([P, 4], f16, tag="u0")
                    nc.vector.tensor_copy(u0sb[:], tpsT[:, :, 0])
                    a2 = spool.tile([P, 4], f16, tag="a2")
                    nc.vector.scalar_tensor_tensor(
                        out=a2[:], in0=u0sb[:], scalar=1.0 / 4096.0,
                        in1=tpsT[:, :, 32], op0=mult, op1=mult,
                    )
                    a3 = spool.tile([P, 4], f16, tag="a3")
                    nc.vector.scalar_tensor_tensor(
                        out=a3[:], in0=a2[:], scalar=k1 / 4096.0,
                        in1=tpsT[:, :, 64], op0=add, op1=mult,
                    )
                    a4 = spool.tile([P, 4], f16, tag="a4")
                    nc.vector.scalar_tensor_tensor(
                        out=a4[:], in0=a3[:], scalar=k2 / 4096.0,
                        in1=tpsT[:, :, 96], op0=add, op1=mult,
                    )
                    # transpose alpha back to one partition ([128,1] -> a
                    # [1,128] slice each), one ACT copy to fp16 SBUF, then
                    # the rank-1 ones broadcast as before
                    aTp = psaT.tile([1, F], f16, tag="aT")
                    for g in range(4):
                        nc.tensor.transpose(
                            aTp[0:1, g * P:(g + 1) * P], a4[:, g:g + 1],
                            id_sb[:],
                        )
                    a4T = abpool.tile([1, F], f16, tag="a4T")
                    nc.scalar.copy(out=a4T[:], in_=aTp[:])
                    abp = psab.tile([P, F], f32, tag="abp")
                    nc.tensor.matmul(
                        abp[:], lhsT=ones128[:], rhs=a4T[:], start=True,
                        stop=True,
                    )
                    ab = abpool.tile([P, 1, F], f16, tag="ab")
                    nc.scalar.activation(
                        ab[:, 0, :], abp[:], Copy, bias=k3 / 4096.0, scale=1.0
                    )

                    # scale + store in halves so the store stream starts
                    # early and stays fed; stores dispatch from Sync, whose
                    # queue is idle once the loads are issued
                    ob = opool.tile([P, NCH, F], f16, tag="o")
                    nc.vector.tensor_tensor(
                        out=ob[:, 0:4, :], in0=xb[:, 0:4, :],
                        in1=ab[:].to_broadcast([P, 4, F]), op=mult,
                    )
                    nc.sync.dma_start(out=out[b, 0], in_=ob[:, 0:4, :])
                    nc.vector.tensor_tensor(
                        out=ob[:, 4:8, :], in0=xb[:, 4:8, :],
                        in1=ab[:].to_broadcast([P, 4, F]), op=mult,
                    )
                    nc.sync.dma_start(out=out[b, 1], in_=ob[:, 4:8, :])

    nc.compile()
    return nc


def _make_in_maps(x, W):
    """Per-core input maps; x [B, D] fp32, W [L, D] fp32."""
    # xt[core, b, h, p, ch, r] = x[core*2048 + b*512 + r, (4h+ch)*128 + p]
    # (partition-major, and each half-block is one contiguous 512KB run)
    xt = np.ascontiguousarray(
        x.reshape(NCORES, NBLK, F, 2, NCH // 2, P).transpose(0, 1, 3, 5, 4, 2)
    ).astype(np.float16)
    wt = np.zeros((P, NCH + 1, M), dtype=np.float16)
    wt[:, :NCH, ::32] = W.reshape(L, NCH, P).transpose(2, 1, 0)
    wt[0, NCH, ::32] = 1.0  # embedded ones-row for the +1 rank-1 update
    idn = np.eye(P, dtype=np.float16)  # PE-transpose identity
    return [{"xt": xt[c], "wtd": wt, "idn": idn} for c in range(NCORES)]


def kernel(x, cross_weights, cross_bias):
    from concourse.bass_utils import run_bass_kernel_spmd

    x = np.ascontiguousarray(np.asarray(x, dtype=np.float32))
    W = np.ascontiguousarray(np.asarray(cross_weights, dtype=np.float32))
    Bb = np.asarray(cross_bias, dtype=np.float32)
    assert x.shape == (BATCH, D) and W.shape == (L, D) and Bb.shape == (L, D)

    # host-side scalar constants k_i = C_i . w_i with C_i = sum_{j<i} b_j
    C = np.zeros(D, dtype=np.float32)
    ks = []
    for i in range(L):
        ks.append(float(C @ W[i]))
        C = C + Bb[i]
    # ks[0] == 0 always (C_0 = 0); bake the other three
    k1, k2, k3 = ks[1], ks[2], ks[3]

    key = (k1, k2, k3)
    nc = _build_cache.get(key)
    if nc is None:
        nc = _build_program(k1, k2, k3)
        _build_cache[key] = nc

    in_maps = _make_in_maps(x, W)
    res = run_bass_kernel_spmd(nc, in_maps, list(range(NCORES)))
    # invert the transposed layout:
    #   full[core*2048 + b*512 + r, (4h+c)*128 + p] = out[core, b, h, p, c, r]
    stacked = np.stack(
        [np.asarray(res.results[c]["out"]) for c in range(NCORES)], axis=0
    ).astype(np.float32)  # [core, b, h, p, ch, F]
    stacked *= 4096.0  # undo the device-side 1/4096 alpha pre-scale
    full = np.ascontiguousarray(
        stacked.transpose(0, 1, 5, 2, 4, 3).reshape(BATCH, D)
    )
    full += C[None, :]  # C_4 broadcast-add on host
    return full
